# revision 9
# baseline (speedup 1.0000x reference)
"""Distributed GQA attention kernel for one TRN2 chip (8 NeuronCores), v3.

Same math/sharding as v2 (tensor-parallel over heads, RoPE via host-split
even/odd weight columns, causal softmax with the V-ones-column denominator
trick, per-head AllToAll, per-token-slice wo projection).

v3 structural changes, aimed at the *marginal* per-iteration cost when the
program is unrolled K times in one NEFF (iterations pipeline through the
same persistent tiles):
  - ALL pools and weights are persistent (allocated once, weights DMA'd
    once).  No pool open/close boundaries between phases or reps, so
    iteration k+1's QKV/x-DMA work is free to overlap iteration k's
    ACT-bound attention tail and collectives (the tile scheduler is
    dependency-driven, not program-order).
  - Flash-style attention loop: per 512-column q-block, compute the <=16
    causal score blocks, exp them into transient [128,512] tiles, and
    immediately consume them with PV.  exp live set drops from 70KB to
    32KB of SBUF, which is what makes everything-persistent fit.
  - PSUM (8 banks): transpose staging 1, Q-real accum 1, {Q-imag,KV}
    accum 2, score chunks 2, {attn-out, wo-accum} shared 2.
"""
from contextlib import ExitStack

import numpy as np

import concourse.bass as bass
import concourse.mybir as mybir
import concourse.tile as tile
from concourse import bacc
from concourse.bass_utils import run_bass_kernel_spmd
from concourse.masks import make_identity

F32 = mybir.dt.float32
BF16 = mybir.dt.bfloat16
AF = mybir.ActivationFunctionType

NC_CORES = 8
B = 2
S = 2048
D = 2048
H = 32
KV = 8
HD = 64
HPC = H // NC_CORES      # 4 q heads per core
EQ = HPC * HD            # 256
T = B * S
TB = 512                 # phase-1 token block
NTB = T // TB
KTILES = S // 128
DT = D // 128
TSLICE = T // NC_CORES
BSL = TSLICE // B        # per-batch token slice each core outputs
QB = 512                 # attention q-block width
NQB = S // QB


def build(reps: int = 1, timeline: bool = False):
    nc = bacc.Bacc("TRN2", target_bir_lowering=False, debug=False,
                   num_devices=NC_CORES)

    x = nc.dram_tensor("x", [T, D], BF16, kind="ExternalInput")
    cos4 = nc.dram_tensor("cos4", [128, S], BF16, kind="ExternalInput")
    sin4 = nc.dram_tensor("sin4", [128, S], BF16, kind="ExternalInput")
    wqTA = nc.dram_tensor("wqTA", [D, 128], BF16, kind="ExternalInput")
    wqTB = nc.dram_tensor("wqTB", [D, 128], BF16, kind="ExternalInput")
    wkvT = nc.dram_tensor("wkvT", [D, 128], BF16, kind="ExternalInput")
    woT = nc.dram_tensor("woT", [D, D], BF16, kind="ExternalInput")
    out = nc.dram_tensor("out", [TSLICE, D], F32, kind="ExternalOutput")

    a2a_in = [nc.dram_tensor(f"a2a_in{b}", [NC_CORES, EQ, BSL], BF16)
              for b in range(B)]
    a2a_out = [nc.dram_tensor(f"a2a_out{b}", [NC_CORES, EQ, BSL], BF16)
               for b in range(B)]
    rg = [list(range(NC_CORES))]

    with tile.TileContext(nc) as tc, ExitStack() as es:
        const = es.enter_context(tc.tile_pool(name="const", bufs=1))
        ident = const.tile([128, 128], BF16, tag="ident")
        make_identity(nc, ident[:])
        # token-major V for all B*KTILES 128-token blocks in 65-col slots;
        # data cols are overwritten every iteration, the ones columns (the
        # softmax-denominator trick) persist from this single memset.
        V_all = const.tile([128, B * KTILES * 65], BF16, tag="vall")
        nc.gpsimd.memset(V_all[:], 1.0)

        qt_pool = es.enter_context(tc.tile_pool(name="qt", bufs=1))
        QTb = [[qt_pool.tile([128, S], BF16, tag=f"QT{b}{g}", name=f"QT{b}{g}")
                for g in range(2)] for b in range(B)]
        KTb = [qt_pool.tile([128, S], BF16, tag=f"KT{b}", name=f"KT{b}")
               for b in range(B)]

        # persistent weights, loaded once
        wsb = es.enter_context(tc.tile_pool(name="wsb", bufs=1))
        cos_sb = wsb.tile([128, S], BF16, tag="cos")
        sin_sb = wsb.tile([128, S], BF16, tag="sin")
        nc.sync.dma_start(cos_sb[:], cos4.ap())
        nc.sync.dma_start(sin_sb[:], sin4.ap())
        wq_sb_A = wsb.tile([128, DT, 128], BF16, tag="wqA")
        wq_sb_B = wsb.tile([128, DT, 128], BF16, tag="wqB")
        wkv_sb = wsb.tile([128, DT, 128], BF16, tag="wkv")
        nc.gpsimd.dma_start(
            wq_sb_A[:], wqTA.ap().rearrange("(dt p) e -> p dt e", p=128))
        nc.gpsimd.dma_start(
            wq_sb_B[:], wqTB.ap().rearrange("(dt p) e -> p dt e", p=128))
        nc.gpsimd.dma_start(
            wkv_sb[:], wkvT.ap().rearrange("(dt p) e -> p dt e", p=128))
        wo_sb = []
        for dt in range(DT):
            w = wsb.tile([128, D], BF16, tag=f"wo{dt}", name=f"wo{dt}")
            nc.gpsimd.dma_start(w[:], woT[128 * dt:128 * (dt + 1), :])
            wo_sb.append(w)

        # persistent working pools
        xbfp = es.enter_context(tc.tile_pool(name="xbfp", bufs=5))
        xtp = es.enter_context(tc.tile_pool(name="xtp", bufs=16))
        ropep = es.enter_context(tc.tile_pool(name="ropep", bufs=1))
        esp = es.enter_context(tc.tile_pool(name="esp", bufs=2))
        att = es.enter_context(tc.tile_pool(name="att", bufs=2))
        rcvp = es.enter_context(tc.tile_pool(name="rcv", bufs=DT))
        p3sb = es.enter_context(tc.tile_pool(name="p3sb", bufs=2))
        pstp = es.enter_context(
            tc.tile_pool(name="pst", bufs=2, space="PSUM"))
        pqp = es.enter_context(
            tc.tile_pool(name="pq", bufs=2, space="PSUM"))
        psSp = es.enter_context(
            tc.tile_pool(name="psS", bufs=2, space="PSUM"))
        miscp = es.enter_context(
            tc.tile_pool(name="misc", bufs=2, space="PSUM"))

        def p1_block(tb):
            """QKV + RoPE for token block tb (3 accumulation passes)."""
            t0 = tb * TB
            bb, c0 = divmod(t0, S)
            xbf = []
            for i in range(4):
                xt_ = xbfp.tile([128, D], BF16, tag="xbf",
                                name=f"xbf{tb}_{i}")
                nc.gpsimd.dma_start(
                    xt_[:], x[t0 + 128 * i: t0 + 128 * (i + 1), :])
                xbf.append(xt_)
            xT = [None] * DT

            def transpose(dt):
                psT = pstp.tile([128, TB], BF16, tag="pst",
                                name=f"psT{tb}_{dt}")
                for i in range(4):
                    nc.tensor.transpose(
                        psT[:, 128 * i: 128 * (i + 1)],
                        xbf[i][:, 128 * dt: 128 * (dt + 1)],
                        ident[:])
                xt_ = xtp.tile([128, TB], BF16, tag="xT",
                               name=f"xT{tb}_{dt}")
                if dt % 2 == 0:
                    nc.scalar.copy(xt_[:], psT[:])
                else:
                    nc.vector.tensor_copy(xt_[:], psT[:])
                xT[dt] = xt_

            def mm_pass(key, w_sb):
                ps = pqp.tile([128, TB], F32, tag="pq", name=f"{key}{tb}")
                for dt in range(DT):
                    if xT[dt] is None:
                        transpose(dt)
                    nc.tensor.matmul(ps[:], w_sb[:, dt, :], xT[dt][:],
                                     start=(dt == 0), stop=(dt == DT - 1))
                return ps

            cs = cos_sb[:, c0:c0 + TB]
            sn = sin_sb[:, c0:c0 + TB]

            qa = mm_pass("qa", wq_sb_A)
            t1 = ropep.tile([128, TB], F32, tag="t1", name=f"t1_{tb}")
            t3 = ropep.tile([128, TB], F32, tag="t3", name=f"t3_{tb}")
            nc.vector.tensor_mul(t1[:], qa[:], cs)
            nc.vector.tensor_mul(t3[:], qa[:], sn)

            qb = mm_pass("qb", wq_sb_B)
            t2 = ropep.tile([128, TB], F32, tag="t2", name=f"t2_{tb}")
            t4 = ropep.tile([128, TB], F32, tag="t4", name=f"t4_{tb}")
            nc.vector.tensor_mul(t2[:], qb[:], sn)
            nc.vector.tensor_mul(t4[:], qb[:], cs)
            Aout = ropep.tile([128, TB], BF16, tag="Ao", name=f"Ao{tb}")
            Bout = ropep.tile([128, TB], BF16, tag="Bo", name=f"Bo{tb}")
            nc.vector.tensor_sub(Aout[:], t1[:], t2[:])
            nc.vector.tensor_add(Bout[:], t3[:], t4[:])
            for h in range(HPC):
                rb = (h % 2) * 64
                nc.vector.tensor_copy(
                    QTb[bb][h // 2][rb:rb + 32, c0:c0 + TB],
                    Aout[32 * h:32 * (h + 1), :])
                nc.vector.tensor_copy(
                    QTb[bb][h // 2][rb + 32:rb + 64, c0:c0 + TB],
                    Bout[32 * h:32 * (h + 1), :])

            kv = mm_pass("kv", wkv_sb)
            kk = ropep.tile([32, 4 * TB], BF16, tag="kk", name=f"kk{tb}")
            nc.vector.tensor_mul(kk[:, 0 * TB:1 * TB], kv[0:32, :], cs[0:32, :])
            nc.vector.tensor_mul(kk[:, 1 * TB:2 * TB], kv[32:64, :], sn[0:32, :])
            nc.vector.tensor_mul(kk[:, 2 * TB:3 * TB], kv[0:32, :], sn[0:32, :])
            nc.vector.tensor_mul(kk[:, 3 * TB:4 * TB], kv[32:64, :], cs[0:32, :])
            nc.vector.tensor_sub(KTb[bb][0:32, c0:c0 + TB],
                                 kk[:, 0 * TB:1 * TB], kk[:, 1 * TB:2 * TB])
            nc.vector.tensor_add(KTb[bb][32:64, c0:c0 + TB],
                                 kk[:, 2 * TB:3 * TB], kk[:, 3 * TB:4 * TB])
            nc.vector.tensor_copy(KTb[bb][64:128, c0:c0 + TB],
                                  KTb[bb][0:64, c0:c0 + TB])
            vst = ropep.tile([64, TB], BF16, tag="vst", name=f"vst{tb}")
            nc.scalar.copy(vst[:], kv[64:128, :])
            psV = pstp.tile([128, 4 * 64], BF16, tag="pst", name=f"psV{tb}")
            for i in range(4):
                nc.tensor.transpose(psV[:, 64 * i:64 * (i + 1)],
                                    vst[:, 128 * i:128 * (i + 1)],
                                    ident[0:64, 0:64])
            slot0 = bb * KTILES + (t0 % S) // 128
            dst = V_all.rearrange("p (k c) -> p k c", c=65)
            nc.scalar.copy(
                dst[:, slot0:slot0 + 4, 0:64],
                psV[:].rearrange("p (k c) -> p k c", c=64))

        def p2_head(b, h):
            """Attention for (b, h), q-block-major (flash-style)."""
            qrows = QTb[b][h // 2][(h % 2) * 64:(h % 2) * 64 + 64, :]
            kbase = (h % 2) * 64
            attnT = att.tile([64, S], BF16, tag="attnT", name=f"attnT{b}{h}")
            for qb in range(NQB):
                b0 = QB * qb
                es_tiles = {}
                for kt in range(4 * qb + 4):
                    off = max(0, 128 * kt - b0)
                    w = QB - off
                    e = esp.tile([128, QB], BF16, tag=f"es{kt}",
                                 bufs=2 if kt < 8 else 1, name=f"es{kt}")
                    es_tiles[kt] = e
                    klhs = KTb[b][kbase:kbase + 64,
                                  128 * kt: 128 * (kt + 1)]
                    ps = psSp.tile([128, QB], F32, tag="psS",
                                   name=f"psS{kt}")
                    nc.tensor.matmul(ps[:, off:QB], klhs,
                                     qrows[:, b0 + off: b0 + QB],
                                     start=True, stop=True)
                    nc.scalar.activation(e[:, off:QB], ps[:, off:QB],
                                         AF.Exp, scale=0.125)
                    if off > 0 or kt == 4 * qb:
                        # diagonal block: zero the strict upper triangle
                        nc.gpsimd.affine_select(
                            out=e[:, off:off + 128], in_=e[:, off:off + 128],
                            compare_op=mybir.AluOpType.is_ge, fill=0.0,
                            base=0, pattern=[[1, 128]], channel_multiplier=-1)
                psO = None
                for qt in range(4 * qb, 4 * qb + 4):
                    if qt % 2 == 0:
                        psO = miscp.tile([128, 130], F32, tag="mx",
                                         name=f"psO{b}{h}{qt}")
                    c0 = 65 * (qt % 2)
                    for i in range(qt + 1):
                        v0 = 65 * (b * KTILES + i)
                        nc.tensor.matmul(
                            psO[:, c0:c0 + 65],
                            es_tiles[i][:, 128 * qt - b0 - 0:
                                        128 * qt - b0 + 128],
                            V_all[:, v0:v0 + 65],
                            start=(i == 0), stop=(i == qt))
                    rc = att.tile([128, 1], F32, tag="rc", name=f"rc{qt}")
                    nc.vector.reciprocal(rc[:], psO[:, c0 + 64:c0 + 65])
                    attn_n = att.tile([128, 64], BF16, tag="attn_n",
                                      name=f"an{qt}")
                    nc.vector.tensor_scalar(attn_n[:], psO[:, c0:c0 + 64],
                                            rc[:], None,
                                            mybir.AluOpType.mult)
                    psAT = miscp.tile([64, 128], BF16, tag="mx",
                                      name=f"psAT{qt}")
                    nc.tensor.transpose(psAT[:], attn_n[:], ident[:])
                    nc.vector.tensor_copy(
                        attnT[:, 128 * qt:128 * (qt + 1)], psAT[:])
            for j in range(NC_CORES):
                nc.sync.dma_start(
                    a2a_in[b][j, HD * h:HD * (h + 1), :],
                    attnT[:, BSL * j:BSL * (j + 1)])

        def collective(b):
            if timeline:
                nc.gpsimd.dma_start(a2a_out[b][:], a2a_in[b][:])
            else:
                nc.gpsimd.collective_compute(
                    "AllToAll", mybir.AluOpType.bypass, replica_groups=rg,
                    ins=[a2a_in[b][:]], outs=[a2a_out[b][:]])

        def p3_batch(b):
            rcv = []
            for dt in range(DT):
                r = rcvp.tile([128, BSL], BF16, tag="rcv",
                              name=f"rcv{b}_{dt}")
                nc.sync.dma_start(
                    r[:],
                    a2a_out[b][dt // 2,
                               (dt % 2) * 128:(dt % 2) * 128 + 128, :])
                rcv.append(r)
            for tt in range(BSL // 128):
                for eb in range(8):
                    psW = miscp.tile([128, 256], F32, tag="mx",
                                     name=f"psW{b}{tt}{eb}")
                    for dt in range(DT):
                        nc.tensor.matmul(
                            psW[:],
                            rcv[dt][:, 128 * tt:128 * (tt + 1)],
                            wo_sb[dt][:, 256 * eb:256 * (eb + 1)],
                            start=(dt == 0), stop=(dt == DT - 1))
                    osb = p3sb.tile([128, 256], F32, tag="osb",
                                    name=f"osb{b}{tt}{eb}")
                    nc.scalar.copy(osb[:], psW[:])
                    nc.sync.dma_start(
                        out[b * BSL + 128 * tt: b * BSL + 128 * (tt + 1),
                            256 * eb:256 * (eb + 1)],
                        osb[:])

        for _rep in range(reps):
            for tb in range(NTB):
                p1_block(tb)
            for h in range(HPC):
                p2_head(0, h)
            collective(0)
            for h in range(HPC):
                p2_head(1, h)
            p3_batch(0)
            collective(1)
            p3_batch(1)

    nc.compile()
    return nc


def _perm_eo(n):
    return list(range(0, n, 2)) + list(range(1, n, 2))


def _bf16(a):
    import ml_dtypes
    return np.ascontiguousarray(np.asarray(a, dtype=np.float32)).astype(
        ml_dtypes.bfloat16)


def host_inputs(x, freqs_cos, freqs_sin, wq, wk, wv, wo):
    x2d = _bf16(np.asarray(x).reshape(T, D))
    fcT = np.asarray(freqs_cos).T.astype(np.float32)
    fsT = np.asarray(freqs_sin).T.astype(np.float32)
    cos4 = _bf16(np.tile(fcT, (4, 1)))
    sin4 = _bf16(np.tile(fsT, (4, 1)))
    woT = _bf16(np.asarray(wo).T)
    wq = np.asarray(wq)
    wk = np.asarray(wk)
    wv = np.asarray(wv)

    permA = [h * HD + 2 * j for h in range(HPC) for j in range(HD // 2)]
    permB = [h * HD + 2 * j + 1 for h in range(HPC) for j in range(HD // 2)]
    permK = _perm_eo(HD)

    in_maps = []
    for c in range(NC_CORES):
        wq_c = wq[EQ * c: EQ * (c + 1)]
        wqTA_ = _bf16(wq_c[permA].T)
        wqTB_ = _bf16(wq_c[permB].T)
        wk_c = wk[HD * c: HD * (c + 1)]
        wv_c = wv[HD * c: HD * (c + 1)]
        wkvT_ = _bf16(np.concatenate([wk_c[permK], wv_c], axis=0).T)
        in_maps.append({
            "x": x2d, "cos4": cos4, "sin4": sin4,
            "wqTA": wqTA_, "wqTB": wqTB_, "wkvT": wkvT_, "woT": woT,
        })
    return in_maps


def host_gather(results):
    full = np.zeros((B, S, D), np.float32)
    for c in range(NC_CORES):
        o = results[c]["out"]
        for b in range(B):
            full[b, BSL * c: BSL * (c + 1), :] = o[b * BSL:(b + 1) * BSL]
    return full


_NC_CACHE = None


def _get_nc():
    global _NC_CACHE
    if _NC_CACHE is None:
        _NC_CACHE = build()
    return _NC_CACHE


def kernel(x, freqs_cos, freqs_sin, wq, wk, wv, wo):
    nc = _get_nc()
    in_maps = host_inputs(x, freqs_cos, freqs_sin, wq, wk, wv, wo)
    res = run_bass_kernel_spmd(nc, in_maps, core_ids=list(range(NC_CORES)))
    return host_gather(res.results)


# revision 10
# speedup vs baseline: 2.3121x; 2.3121x over previous
"""Distributed GQA attention kernel for one TRN2 chip (8 NeuronCores), v3.

Same math/sharding as v2 (tensor-parallel over heads, RoPE via host-split
even/odd weight columns, causal softmax with the V-ones-column denominator
trick, per-head AllToAll, per-token-slice wo projection).

v3 structural changes, aimed at the *marginal* per-iteration cost when the
program is unrolled K times in one NEFF (iterations pipeline through the
same persistent tiles):
  - ALL pools and weights are persistent (allocated once, weights DMA'd
    once).  No pool open/close boundaries between phases or reps, so
    iteration k+1's QKV/x-DMA work is free to overlap iteration k's
    ACT-bound attention tail and collectives (the tile scheduler is
    dependency-driven, not program-order).
  - Flash-style attention loop: per 512-column q-block, compute the <=16
    causal score blocks, exp them into transient [128,512] tiles, and
    immediately consume them with PV.  exp live set drops from 70KB to
    32KB of SBUF, which is what makes everything-persistent fit.
  - PSUM (8 banks): transpose staging 1, Q-real accum 1, {Q-imag,KV}
    accum 2, score chunks 2, {attn-out, wo-accum} shared 2.
"""
from contextlib import ExitStack

import numpy as np

import concourse.bass as bass
import concourse.mybir as mybir
import concourse.tile as tile
from concourse import bacc
from concourse.bass_utils import run_bass_kernel_spmd
from concourse.masks import make_identity

F32 = mybir.dt.float32
BF16 = mybir.dt.bfloat16
AF = mybir.ActivationFunctionType

NC_CORES = 8
B = 2
S = 2048
D = 2048
H = 32
KV = 8
HD = 64
HPC = H // NC_CORES      # 4 q heads per core
EQ = HPC * HD            # 256
T = B * S
TB = 512                 # phase-1 token block
NTB = T // TB
KTILES = S // 128
DT = D // 128
TSLICE = T // NC_CORES
BSL = TSLICE // B        # per-batch token slice each core outputs
QB = 512                 # attention q-block width
NQB = S // QB


def build(reps: int = 1, timeline: bool = False):
    nc = bacc.Bacc("TRN2", target_bir_lowering=False, debug=False,
                   num_devices=NC_CORES)

    x = nc.dram_tensor("x", [T, D], BF16, kind="ExternalInput")
    cos4 = nc.dram_tensor("cos4", [128, S], BF16, kind="ExternalInput")
    sin4 = nc.dram_tensor("sin4", [128, S], BF16, kind="ExternalInput")
    wqTA = nc.dram_tensor("wqTA", [D, 128], BF16, kind="ExternalInput")
    wqTB = nc.dram_tensor("wqTB", [D, 128], BF16, kind="ExternalInput")
    wkvT = nc.dram_tensor("wkvT", [D, 128], BF16, kind="ExternalInput")
    woT = nc.dram_tensor("woT", [D, D], BF16, kind="ExternalInput")
    out = nc.dram_tensor("out", [TSLICE, D], F32, kind="ExternalOutput")

    a2a_in = [nc.dram_tensor(f"a2a_in{b}", [NC_CORES, EQ, BSL], BF16)
              for b in range(B)]
    a2a_out = [nc.dram_tensor(f"a2a_out{b}", [NC_CORES, EQ, BSL], BF16)
               for b in range(B)]
    rg = [list(range(NC_CORES))]

    with tile.TileContext(nc) as tc, ExitStack() as es:
        const = es.enter_context(tc.tile_pool(name="const", bufs=1))
        ident = const.tile([128, 128], BF16, tag="ident")
        make_identity(nc, ident[:])
        # token-major V for all B*KTILES 128-token blocks in 65-col slots;
        # data cols are overwritten every iteration, the ones columns (the
        # softmax-denominator trick) persist from this single memset.
        V_all = const.tile([128, B * KTILES * 65], BF16, tag="vall")
        nc.gpsimd.memset(V_all[:], 1.0)

        qt_pool = es.enter_context(tc.tile_pool(name="qt", bufs=1))
        QTb = [[qt_pool.tile([128, S], BF16, tag=f"QT{b}{g}", name=f"QT{b}{g}")
                for g in range(2)] for b in range(B)]
        KTb = [qt_pool.tile([128, S], BF16, tag=f"KT{b}", name=f"KT{b}")
               for b in range(B)]

        # persistent weights, loaded once
        wsb = es.enter_context(tc.tile_pool(name="wsb", bufs=1))
        cos_sb = wsb.tile([128, S], BF16, tag="cos")
        sin_sb = wsb.tile([128, S], BF16, tag="sin")
        nc.sync.dma_start(cos_sb[:], cos4.ap())
        nc.sync.dma_start(sin_sb[:], sin4.ap())
        wq_sb_A = wsb.tile([128, DT, 128], BF16, tag="wqA")
        wq_sb_B = wsb.tile([128, DT, 128], BF16, tag="wqB")
        wkv_sb = wsb.tile([128, DT, 128], BF16, tag="wkv")
        nc.gpsimd.dma_start(
            wq_sb_A[:], wqTA.ap().rearrange("(dt p) e -> p dt e", p=128))
        nc.gpsimd.dma_start(
            wq_sb_B[:], wqTB.ap().rearrange("(dt p) e -> p dt e", p=128))
        nc.gpsimd.dma_start(
            wkv_sb[:], wkvT.ap().rearrange("(dt p) e -> p dt e", p=128))
        wo_sb = []
        for dt in range(DT):
            w = wsb.tile([128, D], BF16, tag=f"wo{dt}", name=f"wo{dt}")
            nc.gpsimd.dma_start(w[:], woT[128 * dt:128 * (dt + 1), :])
            wo_sb.append(w)

        # persistent working pools
        xbfp = es.enter_context(tc.tile_pool(name="xbfp", bufs=5))
        xtp = es.enter_context(tc.tile_pool(name="xtp", bufs=16))
        ropep = es.enter_context(tc.tile_pool(name="ropep", bufs=1))
        esp = es.enter_context(tc.tile_pool(name="esp", bufs=2))
        att = es.enter_context(tc.tile_pool(name="att", bufs=2))
        rcvp = es.enter_context(tc.tile_pool(name="rcv", bufs=DT))
        p3sb = es.enter_context(tc.tile_pool(name="p3sb", bufs=2))
        pstp = es.enter_context(
            tc.tile_pool(name="pst", bufs=2, space="PSUM"))
        pqp = es.enter_context(
            tc.tile_pool(name="pq", bufs=2, space="PSUM"))
        psSp = es.enter_context(
            tc.tile_pool(name="psS", bufs=2, space="PSUM"))
        miscp = es.enter_context(
            tc.tile_pool(name="misc", bufs=2, space="PSUM"))

        def p1_block(tb):
            """QKV + RoPE for token block tb (3 accumulation passes)."""
            t0 = tb * TB
            bb, c0 = divmod(t0, S)
            xbf = []
            for i in range(4):
                xt_ = xbfp.tile([128, D], BF16, tag="xbf",
                                name=f"xbf{tb}_{i}")
                nc.gpsimd.dma_start(
                    xt_[:], x[t0 + 128 * i: t0 + 128 * (i + 1), :])
                xbf.append(xt_)
            xT = [None] * DT

            def transpose(dt):
                psT = pstp.tile([128, TB], BF16, tag="pst",
                                name=f"psT{tb}_{dt}")
                for i in range(4):
                    nc.tensor.transpose(
                        psT[:, 128 * i: 128 * (i + 1)],
                        xbf[i][:, 128 * dt: 128 * (dt + 1)],
                        ident[:])
                xt_ = xtp.tile([128, TB], BF16, tag="xT",
                               name=f"xT{tb}_{dt}")
                nc.vector.tensor_copy(xt_[:], psT[:])
                xT[dt] = xt_

            def mm_pass(key, w_sb):
                ps = pqp.tile([128, TB], F32, tag="pq", name=f"{key}{tb}")
                for dt in range(DT):
                    if xT[dt] is None:
                        transpose(dt)
                    nc.tensor.matmul(ps[:], w_sb[:, dt, :], xT[dt][:],
                                     start=(dt == 0), stop=(dt == DT - 1))
                return ps

            cs = cos_sb[:, c0:c0 + TB]
            sn = sin_sb[:, c0:c0 + TB]

            qa = mm_pass("qa", wq_sb_A)
            t1 = ropep.tile([128, TB], F32, tag="t1", name=f"t1_{tb}")
            t3 = ropep.tile([128, TB], F32, tag="t3", name=f"t3_{tb}")
            nc.vector.tensor_mul(t1[:], qa[:], cs)
            nc.vector.tensor_mul(t3[:], qa[:], sn)

            qb = mm_pass("qb", wq_sb_B)
            t2 = ropep.tile([128, TB], F32, tag="t2", name=f"t2_{tb}")
            t4 = ropep.tile([128, TB], F32, tag="t4", name=f"t4_{tb}")
            nc.vector.tensor_mul(t2[:], qb[:], sn)
            nc.vector.tensor_mul(t4[:], qb[:], cs)
            Aout = ropep.tile([128, TB], BF16, tag="Ao", name=f"Ao{tb}")
            Bout = ropep.tile([128, TB], BF16, tag="Bo", name=f"Bo{tb}")
            nc.vector.tensor_sub(Aout[:], t1[:], t2[:])
            nc.vector.tensor_add(Bout[:], t3[:], t4[:])
            for h in range(HPC):
                rb = (h % 2) * 64
                nc.vector.tensor_copy(
                    QTb[bb][h // 2][rb:rb + 32, c0:c0 + TB],
                    Aout[32 * h:32 * (h + 1), :])
                nc.vector.tensor_copy(
                    QTb[bb][h // 2][rb + 32:rb + 64, c0:c0 + TB],
                    Bout[32 * h:32 * (h + 1), :])

            kv = mm_pass("kv", wkv_sb)
            kk = ropep.tile([32, 4 * TB], BF16, tag="kk", name=f"kk{tb}")
            nc.vector.tensor_mul(kk[:, 0 * TB:1 * TB], kv[0:32, :], cs[0:32, :])
            nc.vector.tensor_mul(kk[:, 1 * TB:2 * TB], kv[32:64, :], sn[0:32, :])
            nc.vector.tensor_mul(kk[:, 2 * TB:3 * TB], kv[0:32, :], sn[0:32, :])
            nc.vector.tensor_mul(kk[:, 3 * TB:4 * TB], kv[32:64, :], cs[0:32, :])
            nc.vector.tensor_sub(KTb[bb][0:32, c0:c0 + TB],
                                 kk[:, 0 * TB:1 * TB], kk[:, 1 * TB:2 * TB])
            nc.vector.tensor_add(KTb[bb][32:64, c0:c0 + TB],
                                 kk[:, 2 * TB:3 * TB], kk[:, 3 * TB:4 * TB])
            nc.vector.tensor_copy(KTb[bb][64:128, c0:c0 + TB],
                                  KTb[bb][0:64, c0:c0 + TB])
            vst = ropep.tile([64, TB], BF16, tag="vst", name=f"vst{tb}")
            nc.scalar.copy(vst[:], kv[64:128, :])
            psV = pstp.tile([128, 4 * 64], BF16, tag="pst", name=f"psV{tb}")
            for i in range(4):
                nc.tensor.transpose(psV[:, 64 * i:64 * (i + 1)],
                                    vst[:, 128 * i:128 * (i + 1)],
                                    ident[0:64, 0:64])
            slot0 = bb * KTILES + (t0 % S) // 128
            dst = V_all.rearrange("p (k c) -> p k c", c=65)
            nc.scalar.copy(
                dst[:, slot0:slot0 + 4, 0:64],
                psV[:].rearrange("p (k c) -> p k c", c=64))

        def p2_head(b, h):
            """Attention for (b, h), q-block-major (flash-style)."""
            qrows = QTb[b][h // 2][(h % 2) * 64:(h % 2) * 64 + 64, :]
            kbase = (h % 2) * 64
            attnT = att.tile([64, S], BF16, tag="attnT", name=f"attnT{b}{h}")
            for qb in range(NQB):
                b0 = QB * qb
                es_tiles = {}
                for kt in range(4 * qb + 4):
                    off = max(0, 128 * kt - b0)
                    w = QB - off
                    e = esp.tile([128, QB], BF16, tag=f"es{kt}",
                                 bufs=2 if kt < 8 else 1, name=f"es{kt}")
                    es_tiles[kt] = e
                    klhs = KTb[b][kbase:kbase + 64,
                                  128 * kt: 128 * (kt + 1)]
                    ps = psSp.tile([128, QB], F32, tag="psS",
                                   name=f"psS{kt}")
                    nc.tensor.matmul(ps[:, off:QB], klhs,
                                     qrows[:, b0 + off: b0 + QB],
                                     start=True, stop=True)
                    nc.scalar.activation(e[:, off:QB], ps[:, off:QB],
                                         AF.Exp, scale=0.125)
                    if off > 0 or kt == 4 * qb:
                        # diagonal block: zero the strict upper triangle
                        nc.gpsimd.affine_select(
                            out=e[:, off:off + 128], in_=e[:, off:off + 128],
                            compare_op=mybir.AluOpType.is_ge, fill=0.0,
                            base=0, pattern=[[1, 128]], channel_multiplier=-1)
                psO = None
                for qt in range(4 * qb, 4 * qb + 4):
                    if qt % 2 == 0:
                        psO = miscp.tile([128, 130], F32, tag="mx",
                                         name=f"psO{b}{h}{qt}")
                    c0 = 65 * (qt % 2)
                    for i in range(qt + 1):
                        v0 = 65 * (b * KTILES + i)
                        nc.tensor.matmul(
                            psO[:, c0:c0 + 65],
                            es_tiles[i][:, 128 * qt - b0 - 0:
                                        128 * qt - b0 + 128],
                            V_all[:, v0:v0 + 65],
                            start=(i == 0), stop=(i == qt))
                    rc = att.tile([128, 1], F32, tag="rc", name=f"rc{qt}")
                    nc.vector.reciprocal(rc[:], psO[:, c0 + 64:c0 + 65])
                    attn_n = att.tile([128, 64], BF16, tag="attn_n",
                                      name=f"an{qt}")
                    nc.vector.tensor_scalar(attn_n[:], psO[:, c0:c0 + 64],
                                            rc[:], None,
                                            mybir.AluOpType.mult)
                    psAT = miscp.tile([64, 128], BF16, tag="mx",
                                      name=f"psAT{qt}")
                    nc.tensor.transpose(psAT[:], attn_n[:], ident[:])
                    nc.vector.tensor_copy(
                        attnT[:, 128 * qt:128 * (qt + 1)], psAT[:])
            for j in range(NC_CORES):
                nc.sync.dma_start(
                    a2a_in[b][j, HD * h:HD * (h + 1), :],
                    attnT[:, BSL * j:BSL * (j + 1)])

        def collective(b):
            if timeline:
                nc.gpsimd.dma_start(a2a_out[b][:], a2a_in[b][:])
            else:
                nc.gpsimd.collective_compute(
                    "AllToAll", mybir.AluOpType.bypass, replica_groups=rg,
                    ins=[a2a_in[b][:]], outs=[a2a_out[b][:]])

        def p3_batch(b):
            rcv = []
            for dt in range(DT):
                r = rcvp.tile([128, BSL], BF16, tag="rcv",
                              name=f"rcv{b}_{dt}")
                nc.sync.dma_start(
                    r[:],
                    a2a_out[b][dt // 2,
                               (dt % 2) * 128:(dt % 2) * 128 + 128, :])
                rcv.append(r)
            for tt in range(BSL // 128):
                for eb in range(8):
                    psW = miscp.tile([128, 256], F32, tag="mx",
                                     name=f"psW{b}{tt}{eb}")
                    for dt in range(DT):
                        nc.tensor.matmul(
                            psW[:],
                            rcv[dt][:, 128 * tt:128 * (tt + 1)],
                            wo_sb[dt][:, 256 * eb:256 * (eb + 1)],
                            start=(dt == 0), stop=(dt == DT - 1))
                    osb = p3sb.tile([128, 256], F32, tag="osb",
                                    name=f"osb{b}{tt}{eb}")
                    nc.scalar.copy(osb[:], psW[:])
                    nc.sync.dma_start(
                        out[b * BSL + 128 * tt: b * BSL + 128 * (tt + 1),
                            256 * eb:256 * (eb + 1)],
                        osb[:])

        for _rep in range(reps):
            for tb in range(NTB):
                p1_block(tb)
            for h in range(HPC):
                p2_head(0, h)
            collective(0)
            for h in range(HPC):
                p2_head(1, h)
            p3_batch(0)
            collective(1)
            p3_batch(1)

    nc.compile()
    return nc


def _perm_eo(n):
    return list(range(0, n, 2)) + list(range(1, n, 2))


def _bf16(a):
    import ml_dtypes
    return np.ascontiguousarray(np.asarray(a, dtype=np.float32)).astype(
        ml_dtypes.bfloat16)


def host_inputs(x, freqs_cos, freqs_sin, wq, wk, wv, wo):
    x2d = _bf16(np.asarray(x).reshape(T, D))
    fcT = np.asarray(freqs_cos).T.astype(np.float32)
    fsT = np.asarray(freqs_sin).T.astype(np.float32)
    cos4 = _bf16(np.tile(fcT, (4, 1)))
    sin4 = _bf16(np.tile(fsT, (4, 1)))
    woT = _bf16(np.asarray(wo).T)
    wq = np.asarray(wq)
    wk = np.asarray(wk)
    wv = np.asarray(wv)

    permA = [h * HD + 2 * j for h in range(HPC) for j in range(HD // 2)]
    permB = [h * HD + 2 * j + 1 for h in range(HPC) for j in range(HD // 2)]
    permK = _perm_eo(HD)

    in_maps = []
    for c in range(NC_CORES):
        wq_c = wq[EQ * c: EQ * (c + 1)]
        wqTA_ = _bf16(wq_c[permA].T)
        wqTB_ = _bf16(wq_c[permB].T)
        wk_c = wk[HD * c: HD * (c + 1)]
        wv_c = wv[HD * c: HD * (c + 1)]
        wkvT_ = _bf16(np.concatenate([wk_c[permK], wv_c], axis=0).T)
        in_maps.append({
            "x": x2d, "cos4": cos4, "sin4": sin4,
            "wqTA": wqTA_, "wqTB": wqTB_, "wkvT": wkvT_, "woT": woT,
        })
    return in_maps


def host_gather(results):
    full = np.zeros((B, S, D), np.float32)
    for c in range(NC_CORES):
        o = results[c]["out"]
        for b in range(B):
            full[b, BSL * c: BSL * (c + 1), :] = o[b * BSL:(b + 1) * BSL]
    return full


_NC_CACHE = None


def _get_nc():
    global _NC_CACHE
    if _NC_CACHE is None:
        _NC_CACHE = build()
    return _NC_CACHE


def kernel(x, freqs_cos, freqs_sin, wq, wk, wv, wo):
    nc = _get_nc()
    in_maps = host_inputs(x, freqs_cos, freqs_sin, wq, wk, wv, wo)
    res = run_bass_kernel_spmd(nc, in_maps, core_ids=list(range(NC_CORES)))
    return host_gather(res.results)


# revision 11
# speedup vs baseline: 3.6748x; 1.5894x over previous
"""Distributed GQA attention kernel for one TRN2 chip (8 NeuronCores), v3.

Same math/sharding as v2 (tensor-parallel over heads, RoPE via host-split
even/odd weight columns, causal softmax with the V-ones-column denominator
trick, per-head AllToAll, per-token-slice wo projection).

v3 structural changes, aimed at the *marginal* per-iteration cost when the
program is unrolled K times in one NEFF (iterations pipeline through the
same persistent tiles):
  - ALL pools and weights are persistent (allocated once, weights DMA'd
    once).  No pool open/close boundaries between phases or reps, so
    iteration k+1's QKV/x-DMA work is free to overlap iteration k's
    ACT-bound attention tail and collectives (the tile scheduler is
    dependency-driven, not program-order).
  - Flash-style attention loop: per 512-column q-block, compute the <=16
    causal score blocks, exp them into transient [128,512] tiles, and
    immediately consume them with PV.  exp live set drops from 70KB to
    32KB of SBUF, which is what makes everything-persistent fit.
  - PSUM (8 banks): transpose staging 1, Q-real accum 1, {Q-imag,KV}
    accum 2, score chunks 2, {attn-out, wo-accum} shared 2.
"""
from contextlib import ExitStack

import numpy as np

import concourse.bass as bass
import concourse.mybir as mybir
import concourse.tile as tile
from concourse import bacc
from concourse.bass_utils import run_bass_kernel_spmd
from concourse.masks import make_identity

F32 = mybir.dt.float32
BF16 = mybir.dt.bfloat16
AF = mybir.ActivationFunctionType

NC_CORES = 8
B = 2
S = 2048
D = 2048
H = 32
KV = 8
HD = 64
HPC = H // NC_CORES      # 4 q heads per core
EQ = HPC * HD            # 256
T = B * S
TB = 512                 # phase-1 token block
NTB = T // TB
KTILES = S // 128
DT = D // 128
TSLICE = T // NC_CORES
BSL = TSLICE // B        # per-batch token slice each core outputs
QB = 512                 # attention q-block width
NQB = S // QB


def build(reps: int = 1, timeline: bool = False):
    nc = bacc.Bacc("TRN2", target_bir_lowering=False, debug=False,
                   num_devices=NC_CORES)

    x = nc.dram_tensor("x", [T, D], BF16, kind="ExternalInput")
    cos4 = nc.dram_tensor("cos4", [128, S], BF16, kind="ExternalInput")
    sin4 = nc.dram_tensor("sin4", [128, S], BF16, kind="ExternalInput")
    wqTA = nc.dram_tensor("wqTA", [D, 128], BF16, kind="ExternalInput")
    wqTB = nc.dram_tensor("wqTB", [D, 128], BF16, kind="ExternalInput")
    wkvT = nc.dram_tensor("wkvT", [D, 128], BF16, kind="ExternalInput")
    woT = nc.dram_tensor("woT", [D, D], BF16, kind="ExternalInput")
    out = nc.dram_tensor("out", [TSLICE, D], F32, kind="ExternalOutput")

    a2a_in = [[nc.dram_tensor(f"a2a_in{b}_{p}", [NC_CORES, EQ, BSL], BF16)
               for p in range(2)] for b in range(B)]
    a2a_out = [[nc.dram_tensor(f"a2a_out{b}_{p}", [NC_CORES, EQ, BSL], BF16)
                for p in range(2)] for b in range(B)]
    rg = [list(range(NC_CORES))]

    with tile.TileContext(nc) as tc, ExitStack() as es:
        const = es.enter_context(tc.tile_pool(name="const", bufs=1))
        ident = const.tile([128, 128], BF16, tag="ident")
        make_identity(nc, ident[:])
        # token-major V for all B*KTILES 128-token blocks in 65-col slots;
        # data cols are overwritten every iteration, the ones columns (the
        # softmax-denominator trick) persist from this single memset.
        V_all = const.tile([128, B * KTILES * 65], BF16, tag="vall")
        nc.gpsimd.memset(V_all[:], 1.0)

        qt_pool = es.enter_context(tc.tile_pool(name="qt", bufs=1))
        QTb = [[qt_pool.tile([128, S], BF16, tag=f"QT{b}{g}", name=f"QT{b}{g}")
                for g in range(2)] for b in range(B)]
        KTb = [qt_pool.tile([128, S], BF16, tag=f"KT{b}", name=f"KT{b}")
               for b in range(B)]

        # persistent weights, loaded once
        wsb = es.enter_context(tc.tile_pool(name="wsb", bufs=1))
        cos_sb = wsb.tile([128, S], BF16, tag="cos")
        sin_sb = wsb.tile([128, S], BF16, tag="sin")
        nc.sync.dma_start(cos_sb[:], cos4.ap())
        nc.sync.dma_start(sin_sb[:], sin4.ap())
        wq_sb_A = wsb.tile([128, DT, 128], BF16, tag="wqA")
        wq_sb_B = wsb.tile([128, DT, 128], BF16, tag="wqB")
        wkv_sb = wsb.tile([128, DT, 128], BF16, tag="wkv")
        nc.gpsimd.dma_start(
            wq_sb_A[:], wqTA.ap().rearrange("(dt p) e -> p dt e", p=128))
        nc.gpsimd.dma_start(
            wq_sb_B[:], wqTB.ap().rearrange("(dt p) e -> p dt e", p=128))
        nc.gpsimd.dma_start(
            wkv_sb[:], wkvT.ap().rearrange("(dt p) e -> p dt e", p=128))
        wo_sb = []
        for dt in range(DT):
            w = wsb.tile([128, D], BF16, tag=f"wo{dt}", name=f"wo{dt}")
            nc.gpsimd.dma_start(w[:], woT[128 * dt:128 * (dt + 1), :])
            wo_sb.append(w)

        # persistent working pools
        xbfp = es.enter_context(tc.tile_pool(name="xbfp", bufs=5))
        xtp = es.enter_context(tc.tile_pool(name="xtp", bufs=16))
        ropep = es.enter_context(tc.tile_pool(name="ropep", bufs=1))
        esp = es.enter_context(tc.tile_pool(name="esp", bufs=2))
        att = es.enter_context(tc.tile_pool(name="att", bufs=2))
        rcvp = es.enter_context(tc.tile_pool(name="rcv", bufs=DT))
        p3sb = es.enter_context(tc.tile_pool(name="p3sb", bufs=2))
        pstp = es.enter_context(
            tc.tile_pool(name="pst", bufs=2, space="PSUM"))
        pqp = es.enter_context(
            tc.tile_pool(name="pq", bufs=2, space="PSUM"))
        psSp = es.enter_context(
            tc.tile_pool(name="psS", bufs=2, space="PSUM"))
        miscp = es.enter_context(
            tc.tile_pool(name="misc", bufs=2, space="PSUM"))

        def p1_block(tb):
            """QKV + RoPE for token block tb (3 accumulation passes)."""
            t0 = tb * TB
            bb, c0 = divmod(t0, S)
            xbf = []
            for i in range(4):
                xt_ = xbfp.tile([128, D], BF16, tag="xbf",
                                name=f"xbf{tb}_{i}")
                nc.gpsimd.dma_start(
                    xt_[:], x[t0 + 128 * i: t0 + 128 * (i + 1), :])
                xbf.append(xt_)
            xT = [None] * DT

            def transpose(dt):
                psT = pstp.tile([128, TB], BF16, tag="pst",
                                name=f"psT{tb}_{dt}")
                for i in range(4):
                    nc.tensor.transpose(
                        psT[:, 128 * i: 128 * (i + 1)],
                        xbf[i][:, 128 * dt: 128 * (dt + 1)],
                        ident[:])
                xt_ = xtp.tile([128, TB], BF16, tag="xT",
                               name=f"xT{tb}_{dt}")
                nc.vector.tensor_copy(xt_[:], psT[:])
                xT[dt] = xt_

            def mm_pass(key, w_sb):
                ps = pqp.tile([128, TB], F32, tag="pq", name=f"{key}{tb}")
                for dt in range(DT):
                    if xT[dt] is None:
                        transpose(dt)
                    nc.tensor.matmul(ps[:], w_sb[:, dt, :], xT[dt][:],
                                     start=(dt == 0), stop=(dt == DT - 1))
                return ps

            cs = cos_sb[:, c0:c0 + TB]
            sn = sin_sb[:, c0:c0 + TB]

            qa = mm_pass("qa", wq_sb_A)
            t1 = ropep.tile([128, TB], F32, tag="t1", name=f"t1_{tb}")
            t3 = ropep.tile([128, TB], F32, tag="t3", name=f"t3_{tb}")
            nc.vector.tensor_mul(t1[:], qa[:], cs)
            nc.vector.tensor_mul(t3[:], qa[:], sn)

            qb = mm_pass("qb", wq_sb_B)
            t2 = ropep.tile([128, TB], F32, tag="t2", name=f"t2_{tb}")
            t4 = ropep.tile([128, TB], F32, tag="t4", name=f"t4_{tb}")
            nc.vector.tensor_mul(t2[:], qb[:], sn)
            nc.vector.tensor_mul(t4[:], qb[:], cs)
            Aout = ropep.tile([128, TB], BF16, tag="Ao", name=f"Ao{tb}")
            Bout = ropep.tile([128, TB], BF16, tag="Bo", name=f"Bo{tb}")
            nc.vector.tensor_sub(Aout[:], t1[:], t2[:])
            nc.vector.tensor_add(Bout[:], t3[:], t4[:])
            for h in range(HPC):
                rb = (h % 2) * 64
                nc.vector.tensor_copy(
                    QTb[bb][h // 2][rb:rb + 32, c0:c0 + TB],
                    Aout[32 * h:32 * (h + 1), :])
                nc.vector.tensor_copy(
                    QTb[bb][h // 2][rb + 32:rb + 64, c0:c0 + TB],
                    Bout[32 * h:32 * (h + 1), :])

            kv = mm_pass("kv", wkv_sb)
            kk = ropep.tile([32, 4 * TB], BF16, tag="kk", name=f"kk{tb}")
            nc.vector.tensor_mul(kk[:, 0 * TB:1 * TB], kv[0:32, :], cs[0:32, :])
            nc.vector.tensor_mul(kk[:, 1 * TB:2 * TB], kv[32:64, :], sn[0:32, :])
            nc.vector.tensor_mul(kk[:, 2 * TB:3 * TB], kv[0:32, :], sn[0:32, :])
            nc.vector.tensor_mul(kk[:, 3 * TB:4 * TB], kv[32:64, :], cs[0:32, :])
            nc.vector.tensor_sub(KTb[bb][0:32, c0:c0 + TB],
                                 kk[:, 0 * TB:1 * TB], kk[:, 1 * TB:2 * TB])
            nc.vector.tensor_add(KTb[bb][32:64, c0:c0 + TB],
                                 kk[:, 2 * TB:3 * TB], kk[:, 3 * TB:4 * TB])
            nc.vector.tensor_copy(KTb[bb][64:128, c0:c0 + TB],
                                  KTb[bb][0:64, c0:c0 + TB])
            vst = ropep.tile([64, TB], BF16, tag="vst", name=f"vst{tb}")
            nc.scalar.copy(vst[:], kv[64:128, :])
            psV = pstp.tile([128, 4 * 64], BF16, tag="pst", name=f"psV{tb}")
            for i in range(4):
                nc.tensor.transpose(psV[:, 64 * i:64 * (i + 1)],
                                    vst[:, 128 * i:128 * (i + 1)],
                                    ident[0:64, 0:64])
            slot0 = bb * KTILES + (t0 % S) // 128
            dst = V_all.rearrange("p (k c) -> p k c", c=65)
            nc.scalar.copy(
                dst[:, slot0:slot0 + 4, 0:64],
                psV[:].rearrange("p (k c) -> p k c", c=64))

        def p2_head(b, h):
            """Attention for (b, h), q-block-major (flash-style)."""
            qrows = QTb[b][h // 2][(h % 2) * 64:(h % 2) * 64 + 64, :]
            kbase = (h % 2) * 64
            attnT = att.tile([64, S], BF16, tag="attnT", name=f"attnT{b}{h}")
            for qb in range(NQB):
                b0 = QB * qb
                es_tiles = {}
                for kt in range(4 * qb + 4):
                    off = max(0, 128 * kt - b0)
                    w = QB - off
                    e = esp.tile([128, QB], BF16, tag=f"es{kt}",
                                 bufs=2 if kt < 8 else 1, name=f"es{kt}")
                    es_tiles[kt] = e
                    klhs = KTb[b][kbase:kbase + 64,
                                  128 * kt: 128 * (kt + 1)]
                    ps = psSp.tile([128, QB], F32, tag="psS",
                                   name=f"psS{kt}")
                    nc.tensor.matmul(ps[:, off:QB], klhs,
                                     qrows[:, b0 + off: b0 + QB],
                                     start=True, stop=True)
                    nc.scalar.activation(e[:, off:QB], ps[:, off:QB],
                                         AF.Exp, scale=0.125)
                    if off > 0 or kt == 4 * qb:
                        # diagonal block: zero the strict upper triangle
                        nc.gpsimd.affine_select(
                            out=e[:, off:off + 128], in_=e[:, off:off + 128],
                            compare_op=mybir.AluOpType.is_ge, fill=0.0,
                            base=0, pattern=[[1, 128]], channel_multiplier=-1)
                psO = None
                for qt in range(4 * qb, 4 * qb + 4):
                    if qt % 2 == 0:
                        psO = miscp.tile([128, 130], F32, tag="mx",
                                         name=f"psO{b}{h}{qt}")
                    c0 = 65 * (qt % 2)
                    for i in range(qt + 1):
                        v0 = 65 * (b * KTILES + i)
                        nc.tensor.matmul(
                            psO[:, c0:c0 + 65],
                            es_tiles[i][:, 128 * qt - b0 - 0:
                                        128 * qt - b0 + 128],
                            V_all[:, v0:v0 + 65],
                            start=(i == 0), stop=(i == qt))
                    rc = att.tile([128, 1], F32, tag="rc", name=f"rc{qt}")
                    nc.vector.reciprocal(rc[:], psO[:, c0 + 64:c0 + 65])
                    attn_n = att.tile([128, 64], BF16, tag="attn_n",
                                      name=f"an{qt}")
                    nc.vector.tensor_scalar(attn_n[:], psO[:, c0:c0 + 64],
                                            rc[:], None,
                                            mybir.AluOpType.mult)
                    psAT = miscp.tile([64, 128], BF16, tag="mx",
                                      name=f"psAT{qt}")
                    nc.tensor.transpose(psAT[:], attn_n[:], ident[:])
                    nc.vector.tensor_copy(
                        attnT[:, 128 * qt:128 * (qt + 1)], psAT[:])
            for j in range(NC_CORES):
                nc.sync.dma_start(
                    a2a_in[b][par][j, HD * h:HD * (h + 1), :],
                    attnT[:, BSL * j:BSL * (j + 1)])

        def collective(b):
            if timeline:
                nc.gpsimd.dma_start(a2a_out[b][par][:], a2a_in[b][par][:])
            else:
                nc.gpsimd.collective_compute(
                    "AllToAll", mybir.AluOpType.bypass, replica_groups=rg,
                    ins=[a2a_in[b][par][:]], outs=[a2a_out[b][par][:]])

        def p3_batch(b):
            rcv = []
            for dt in range(DT):
                r = rcvp.tile([128, BSL], BF16, tag="rcv",
                              name=f"rcv{b}_{dt}")
                nc.sync.dma_start(
                    r[:],
                    a2a_out[b][par][dt // 2,
                                    (dt % 2) * 128:(dt % 2) * 128 + 128, :])
                rcv.append(r)
            for tt in range(BSL // 128):
                for eb in range(8):
                    psW = miscp.tile([128, 256], F32, tag="mx",
                                     name=f"psW{b}{tt}{eb}")
                    for dt in range(DT):
                        nc.tensor.matmul(
                            psW[:],
                            rcv[dt][:, 128 * tt:128 * (tt + 1)],
                            wo_sb[dt][:, 256 * eb:256 * (eb + 1)],
                            start=(dt == 0), stop=(dt == DT - 1))
                    osb = p3sb.tile([128, 256], F32, tag="osb",
                                    name=f"osb{b}{tt}{eb}")
                    nc.scalar.copy(osb[:], psW[:])
                    nc.sync.dma_start(
                        out[b * BSL + 128 * tt: b * BSL + 128 * (tt + 1),
                            256 * eb:256 * (eb + 1)],
                        osb[:])

        par = 0
        for _rep in range(reps):
            par = _rep % 2
            for tb in range(NTB):
                p1_block(tb)
            for h in range(HPC):
                p2_head(0, h)
            collective(0)
            for h in range(HPC):
                p2_head(1, h)
            p3_batch(0)
            collective(1)
            p3_batch(1)

    nc.compile()
    return nc


def _perm_eo(n):
    return list(range(0, n, 2)) + list(range(1, n, 2))


def _bf16(a):
    import ml_dtypes
    return np.ascontiguousarray(np.asarray(a, dtype=np.float32)).astype(
        ml_dtypes.bfloat16)


def host_inputs(x, freqs_cos, freqs_sin, wq, wk, wv, wo):
    x2d = _bf16(np.asarray(x).reshape(T, D))
    fcT = np.asarray(freqs_cos).T.astype(np.float32)
    fsT = np.asarray(freqs_sin).T.astype(np.float32)
    cos4 = _bf16(np.tile(fcT, (4, 1)))
    sin4 = _bf16(np.tile(fsT, (4, 1)))
    woT = _bf16(np.asarray(wo).T)
    wq = np.asarray(wq)
    wk = np.asarray(wk)
    wv = np.asarray(wv)

    permA = [h * HD + 2 * j for h in range(HPC) for j in range(HD // 2)]
    permB = [h * HD + 2 * j + 1 for h in range(HPC) for j in range(HD // 2)]
    permK = _perm_eo(HD)

    in_maps = []
    for c in range(NC_CORES):
        wq_c = wq[EQ * c: EQ * (c + 1)]
        wqTA_ = _bf16(wq_c[permA].T)
        wqTB_ = _bf16(wq_c[permB].T)
        wk_c = wk[HD * c: HD * (c + 1)]
        wv_c = wv[HD * c: HD * (c + 1)]
        wkvT_ = _bf16(np.concatenate([wk_c[permK], wv_c], axis=0).T)
        in_maps.append({
            "x": x2d, "cos4": cos4, "sin4": sin4,
            "wqTA": wqTA_, "wqTB": wqTB_, "wkvT": wkvT_, "woT": woT,
        })
    return in_maps


def host_gather(results):
    full = np.zeros((B, S, D), np.float32)
    for c in range(NC_CORES):
        o = results[c]["out"]
        for b in range(B):
            full[b, BSL * c: BSL * (c + 1), :] = o[b * BSL:(b + 1) * BSL]
    return full


_NC_CACHE = None


def _get_nc():
    global _NC_CACHE
    if _NC_CACHE is None:
        _NC_CACHE = build()
    return _NC_CACHE


def kernel(x, freqs_cos, freqs_sin, wq, wk, wv, wo):
    nc = _get_nc()
    in_maps = host_inputs(x, freqs_cos, freqs_sin, wq, wk, wv, wo)
    res = run_bass_kernel_spmd(nc, in_maps, core_ids=list(range(NC_CORES)))
    return host_gather(res.results)
